# revision 20
# baseline (speedup 1.0000x reference)
"""CourierEncoder fused kernel for 8 Trainium2 NeuronCores.

Data-parallel over the batch: each core processes B/8 = 32768 rows.
Per 512-row tile (matmuls bf16 -> fp32 PSUM), software-pipelined over a
3-stage skew (embeds for tile a=k, layer 1 for b=k-1, layer 2 for c=k-2):
  embeds:  x/y as K=3 outer products {w, b_hi, b_lo} (x) {coord, 1, 1} at
           row strips 0/32 (concurrent via tile_position auto-derive);
           cos folded as Sin(z+pi/2); biases folded into the matmul so both
           Sin activations fuse into ONE scalar-engine op (FD=1024).
           t-embed via host-broadcast tb[128,R] + DVE tensor_scalar
           (per-partition w_t/b_t) + stt LeakyReLU -- no PE, no PSUM.
  b2 bias: ones (x) b2hi/lo matmuls at row strip 96/64, concurrent with
           the x/y embed matmuls (distinct 32-row strips).
  layer 1: feature-major, 6 matmuls [128,128]@[128,512]; bias+LeakyReLU on
           DVE via tensor_scalar_add (fp32 per-partition b1) + SBUF stt
  layer 2: batch-major (lhsT = h1T slices); LeakyReLU on ACT via one
           Prelu op (FD=1024, single PSUM input)
"""

import math

import numpy as np
import ml_dtypes

import concourse.bass as bass
import concourse.tile as tile
import concourse.mybir as mybir
from concourse import bacc
from concourse.bass_utils import run_bass_kernel_spmd

B = 262144
NCORES = 8
R = B // NCORES          # rows per core
TILE = 512               # rows per tile
NT = R // TILE           # tiles per core
G = 4                    # tiles per input DMA group
PED = 256
NED = 128
CED = 256
Q = PED // 4             # 64
ALPHA = 0.01

F32 = mybir.dt.float32
BF16 = mybir.dt.bfloat16
AF = mybir.ActivationFunctionType
ALU = mybir.AluOpType

_CACHE = {}


def _build():
    nc = bacc.Bacc()
    coords = nc.dram_tensor("coords", [6, R], BF16, kind="ExternalInput")
    tb = nc.dram_tensor("tb", [128, R], BF16, kind="ExternalInput")
    embw = nc.dram_tensor("embw", [35, 128], BF16, kind="ExternalInput")
    svec = nc.dram_tensor("svec", [128, 4], F32, kind="ExternalInput")
    w1p = nc.dram_tensor("w1p", [128, 3, 2, 128], BF16, kind="ExternalInput")
    w2p = nc.dram_tensor("w2p", [128, 2, 256], BF16, kind="ExternalInput")
    b2pack = nc.dram_tensor("b2pack", [2, 640], BF16, kind="ExternalInput")
    out = nc.dram_tensor("out", [R, 256], F32, kind="ExternalOutput")

    with tile.TileContext(nc) as tc:
        with (
            tc.tile_pool(name="const", bufs=1) as const,
            tc.tile_pool(name="io", bufs=2) as io,
            tc.tile_pool(name="acts", bufs=4) as acts,
            tc.tile_pool(name="outp", bufs=4) as outp,
            tc.tile_pool(name="ps_emb", bufs=1, space="PSUM") as ps_emb,
            tc.tile_pool(name="ps_l1a", bufs=1, space="PSUM") as ps_l1a,
            tc.tile_pool(name="ps_l1b", bufs=1, space="PSUM") as ps_l1b,
            tc.tile_pool(name="ps_l2", bufs=2, space="PSUM") as ps_l2,
        ):
            embw_sb = const.tile([35, 128], BF16)
            sv_sb = const.tile([128, 4], F32)   # wt, bt, b1c0, b1c1
            w1_sb = const.tile([128, 3, 2, 128], BF16)
            w2_sb = const.tile([128, 2, 256], BF16)
            b2_sb = const.tile([98, 640], BF16)
            nc.sync.dma_start(out=embw_sb, in_=embw[:, :])
            nc.sync.dma_start(out=sv_sb, in_=svec[:, :])
            nc.sync.dma_start(out=b2_sb[96:98, :], in_=b2pack[:, :])
            nc.sync.dma_start(out=b2_sb[64:66, :], in_=b2pack[:, :])

            xyin = [None] * (NT // G)
            tin = [None] * (NT // G)
            # group-0 inputs issue before the (later-needed) w1/w2 weights
            xyin[0] = io.tile([35, G, 512], BF16, tag="xyin", name="xyin0")
            tin[0] = io.tile([128, G, 512], BF16, tag="tin", name="tin0")
            for cc in range(2):
                nc.sync.dma_start(
                    out=xyin[0][32 * cc:32 * cc + 3, :, :],
                    in_=coords[3 * cc:3 * cc + 3, 0:G * 512].rearrange(
                        "p (g n) -> p g n", n=512),
                )
            nc.sync.dma_start(
                out=tin[0],
                in_=tb[:, 0:G * 512].rearrange("p (g n) -> p g n", n=512),
            )
            # weight loads on the (idle) gpsimd SWDGE queue so the sync
            # queue's completion counter ends at the group-0 inputs
            nc.gpsimd.dma_start(out=w1_sb, in_=w1p[:, :, :, :])
            nc.gpsimd.dma_start(out=w2_sb, in_=w2p[:, :, :])
            hxy = [None] * NT
            ht_ = [None] * NT
            h1T = [None] * NT
            l1ps = [None] * NT
            l2ps = [None] * NT

            for k in range(NT + 2):
                a = k          # stage A: embeds
                b = k - 1      # stage B: layer 1
                c = k - 2      # stage C: layer 2 + store

                if a < NT:
                    ga, ja = divmod(a, G)
                    if ja == 0 and ga > 0:
                        lo, hi = ga * G * 512, (ga + 1) * G * 512
                        xyin[ga] = io.tile([35, G, 512], BF16, tag="xyin", name="xyin")
                        tin[ga] = io.tile([128, G, 512], BF16, tag="tin", name="tin")
                        for cc in range(2):
                            nc.sync.dma_start(
                                out=xyin[ga][32 * cc:32 * cc + 3, :, :],
                                in_=coords[3 * cc:3 * cc + 3, lo:hi].rearrange(
                                    "p (g n) -> p g n", n=512),
                            )
                        nc.sync.dma_start(
                            out=tin[ga],
                            in_=tb[:, lo:hi].rearrange("p (g n) -> p g n", n=512),
                        )

                    # strip matmuls: x-emb(0), y-emb(32), b2 bias(64, 96)
                    if c >= 0:
                        l2ps[c] = ps_l2.tile([128, 4, 256], F32, tag="l2", name="l2ps")
                        nc.tensor.matmul(
                            l2ps[c][:, 0:2, :],
                            b2_sb[96:98, 0:128], b2_sb[96:98, 128:640],
                            start=True, stop=False,
                            skip_group_check=True, tile_position=(96, 0),
                        )
                        nc.tensor.matmul(
                            l2ps[c][:, 2:4, :],
                            b2_sb[64:66, 0:128], b2_sb[64:66, 128:640],
                            start=True, stop=False,
                            skip_group_check=True, tile_position=(64, 0),
                        )
                    emb_ps = ps_emb.tile([128, 2, 512], F32)
                    for cc in range(2):
                        nc.tensor.matmul(
                            emb_ps[:, cc, :],
                            embw_sb[32 * cc:32 * cc + 3, :],
                            xyin[ga][32 * cc:32 * cc + 3, ja, :],
                            start=True, stop=True,
                        )
                    hxy[a] = acts.tile([128, 2, 512], BF16, tag="hxy", name="hxy")
                    nc.scalar.activation(out=hxy[a], in_=emb_ps, func=AF.Sin)
                elif c >= 0:
                    l2ps[c] = ps_l2.tile([128, 4, 256], F32, tag="l2", name="l2ps")
                    for h in range(2):
                        nc.tensor.matmul(
                            l2ps[c][:, 2 * h:2 * h + 2, :],
                            b2_sb[96:98, 0:128], b2_sb[96:98, 128:640],
                            start=True, stop=False,
                            skip_group_check=True, tile_position=(96, 0),
                        )

                # -- stage B: layer 1 (feature-major) -----------------------
                if 0 <= b < NT:
                    l1a = ps_l1a.tile([128, 512], F32, name="l1a")
                    l1b = ps_l1b.tile([128, 512], F32, name="l1b")
                    l1ps[b] = (l1a, l1b)
                    for mc, lp in ((0, l1a), (1, l1b)):
                        for kc in range(2):
                            nc.tensor.matmul(
                                lp,
                                w1_sb[:, kc, mc, :],
                                hxy[b][:, kc, :],
                                start=(kc == 0), stop=False,
                            )
                        nc.tensor.matmul(
                            lp,
                            w1_sb[:, 2, mc, :],
                            ht_[b],
                            start=False, stop=True,
                        )
                    h1T[b] = acts.tile([128, 2, 512], BF16, tag="h1T", name="h1T")
                    # mc1 + mc0-head on DVE, mc0-tail on ACT (engine balance)
                    tmp = acts.tile([128, 512], BF16, tag="tmp1")
                    nc.vector.tensor_scalar_add(
                        out=tmp, in0=l1ps[b][1],
                        scalar1=sv_sb[:, 3:4])
                    nc.vector.scalar_tensor_tensor(
                        out=h1T[b][:, 1, :], in0=tmp, scalar=ALPHA,
                        in1=tmp, op0=ALU.mult, op1=ALU.max)
                    nc.scalar.activation(out=h1T[b][:, 0, :],
                                         in_=l1ps[b][0],
                                         func=AF.Prelu, bias=sv_sb[:, 2:3],
                                         alpha=ALPHA)

                # t-embed on DVE from broadcast tb (issued after mc1 so the
                # l1_ps reader runs at DVE queue head)
                if a < NT:
                    zt = acts.tile([128, 512], BF16, tag="zt")
                    ht_[a] = acts.tile([128, 512], BF16, tag="ht", name="ht")
                    nc.vector.tensor_scalar(
                        out=zt, in0=tin[ga][:, ja, :],
                        scalar1=sv_sb[:, 0:1], scalar2=sv_sb[:, 1:2],
                        op0=ALU.mult, op1=ALU.add)
                    nc.vector.scalar_tensor_tensor(
                        out=ht_[a], in0=zt, scalar=ALPHA, in1=zt,
                        op0=ALU.mult, op1=ALU.max)

                # -- stage C: layer 2 (batch-major) + LeakyReLU + store -----
                if c >= 0:
                    for r in range(4):
                        for kc in range(2):
                            nc.tensor.matmul(
                                l2ps[c][:, r, :],
                                h1T[c][:, kc, r * 128:(r + 1) * 128],
                                w2_sb[:, kc, :],
                                start=False, stop=(kc == 1),
                                skip_group_check=True,
                            )
                    o_sb = outp.tile([128, 4, 256], F32)
                    nc.scalar.activation(out=o_sb, in_=l2ps[c],
                                         func=AF.Prelu, alpha=ALPHA)
                    base = c * TILE
                    nc.sync.dma_start(
                        out=out[base:base + TILE, :].rearrange(
                            "(r p) m -> p r m", p=128),
                        in_=o_sb,
                    )
                    hxy[c] = ht_[c] = h1T[c] = l1ps[c] = l2ps[c] = None
    nc.finalize()
    return nc


def _prep_weights(inputs):
    f = {k: np.asarray(v, dtype=np.float32) for k, v in inputs.items()}
    bf = ml_dtypes.bfloat16

    def hilo(v):
        hi = v.astype(bf).astype(np.float32)
        return hi.astype(bf), (v - hi).astype(bf)

    embw = np.zeros((35, 128), bf)
    embw[0] = np.concatenate([f["w_sx"].ravel(), f["w_cx"].ravel()])
    bx = np.concatenate([f["b_sx"], f["b_cx"] + math.pi / 2])
    embw[1], embw[2] = hilo(bx)
    embw[32] = np.concatenate([f["w_sy"].ravel(), f["w_cy"].ravel()])
    by = np.concatenate([f["b_sy"], f["b_cy"] + math.pi / 2])
    embw[33], embw[34] = hilo(by)

    svec = np.stack([
        np.repeat(f["w_t"].ravel(), 1),
        f["b_t"],
        f["b1"][0:128],
        f["b1"][128:256],
    ], axis=1)
    svec = np.ascontiguousarray(svec, dtype=np.float32)

    w1p = f["w1"].reshape(3, 128, 2, 128).transpose(1, 0, 2, 3).astype(bf)
    w2p = f["w2"].reshape(2, 128, 256).transpose(1, 0, 2).astype(bf)

    b2hi, b2lo = hilo(f["b2"])
    b2pack = np.zeros((2, 640), bf)
    b2pack[:, 0:128] = 1.0
    b2pack[0, 128:640] = np.concatenate([b2hi, b2hi])
    b2pack[1, 128:640] = np.concatenate([b2lo, b2lo])

    return {
        "embw": embw,
        "svec": svec,
        "w1p": np.ascontiguousarray(w1p),
        "w2p": np.ascontiguousarray(w2p),
        "b2pack": b2pack,
    }


def kernel(**inputs):
    if "nc" not in _CACHE:
        _CACHE["nc"] = _build()
    nc = _CACHE["nc"]

    w = _prep_weights(inputs)
    bf = ml_dtypes.bfloat16
    xy = np.asarray(inputs["xy"], dtype=np.float32)
    t = np.asarray(inputs["t"], dtype=np.float32)

    coords = np.empty((6, B), bf)
    coords[0] = xy[:, 0].astype(bf)
    coords[1:3] = 1.0
    coords[3] = xy[:, 1].astype(bf)
    coords[4:6] = 1.0
    t_bf = t[:, 0].astype(bf)

    in_maps = []
    for c in range(NCORES):
        lo, hi = c * R, (c + 1) * R
        in_maps.append({
            "coords": np.ascontiguousarray(coords[:, lo:hi]),
            "tb": np.ascontiguousarray(
                np.broadcast_to(t_bf[lo:hi], (128, R))),
            **w,
        })

    res = run_bass_kernel_spmd(nc, in_maps, core_ids=list(range(NCORES)))
    _CACHE["last_res"] = res
    return np.concatenate([res.results[c]["out"] for c in range(NCORES)], axis=0)


# revision 22
# speedup vs baseline: 1.0079x; 1.0079x over previous
"""CourierEncoder fused kernel for 8 Trainium2 NeuronCores.

Data-parallel over the batch: each core processes B/8 = 32768 rows.
Per 512-row tile (matmuls bf16 -> fp32 PSUM), software-pipelined over a
3-stage skew (embeds for tile a=k, layer 1 for b=k-1, layer 2 for c=k-2):
  embeds:  x/y as K=3 outer products {w, b_hi, b_lo} (x) {coord, 1, 1} at
           row strips 0/32 (concurrent via tile_position auto-derive);
           cos folded as Sin(z+pi/2); biases folded into the matmul so both
           Sin activations fuse into ONE scalar-engine op (FD=1024).
           t-embed via host-broadcast tb[128,R] + DVE tensor_scalar
           (per-partition w_t/b_t) + stt LeakyReLU -- no PE, no PSUM.
  b2 bias: ones (x) b2hi/lo matmuls at row strip 96/64, concurrent with
           the x/y embed matmuls (distinct 32-row strips).
  layer 1: feature-major, 6 matmuls [128,128]@[128,512]; bias+LeakyReLU on
           DVE via tensor_scalar_add (fp32 per-partition b1) + SBUF stt
  layer 2: batch-major (lhsT = h1T slices); LeakyReLU on ACT via one
           Prelu op (FD=1024, single PSUM input)
"""

import math

import numpy as np
import ml_dtypes

import concourse.bass as bass
import concourse.tile as tile
import concourse.mybir as mybir
from concourse import bacc
from concourse.bass_utils import run_bass_kernel_spmd

B = 262144
NCORES = 8
R = B // NCORES          # rows per core
TILE = 512               # rows per tile
NT = R // TILE           # tiles per core
G = 4                    # tiles per input DMA group
PED = 256
NED = 128
CED = 256
Q = PED // 4             # 64
ALPHA = 0.01

F32 = mybir.dt.float32
BF16 = mybir.dt.bfloat16
AF = mybir.ActivationFunctionType
ALU = mybir.AluOpType

_CACHE = {}


def _build():
    nc = bacc.Bacc()
    coords = nc.dram_tensor("coords", [6, R], BF16, kind="ExternalInput")
    tb = nc.dram_tensor("tb", [128, R], BF16, kind="ExternalInput")
    cpack = nc.dram_tensor("cpack", [128, 776], BF16, kind="ExternalInput")
    w1p = nc.dram_tensor("w1p", [128, 3, 2, 128], BF16, kind="ExternalInput")
    w2p = nc.dram_tensor("w2p", [128, 2, 256], BF16, kind="ExternalInput")
    out = nc.dram_tensor("out", [R, 256], F32, kind="ExternalOutput")

    with tile.TileContext(nc) as tc:
        with (
            tc.tile_pool(name="const", bufs=1) as const,
            tc.tile_pool(name="io", bufs=2) as io,
            tc.tile_pool(name="acts", bufs=4) as acts,
            tc.tile_pool(name="outp", bufs=4) as outp,
            tc.tile_pool(name="ps_emb", bufs=1, space="PSUM") as ps_emb,
            tc.tile_pool(name="ps_l1a", bufs=1, space="PSUM") as ps_l1a,
            tc.tile_pool(name="ps_l1b", bufs=1, space="PSUM") as ps_l1b,
            tc.tile_pool(name="ps_l2", bufs=2, space="PSUM") as ps_l2,
        ):
            # single packed const tile: embw rows 0-34 cols 0:128; svec (f32
            # bit-cast) cols 128:136; b2 ones+hi/lo at partitions 64/96 cols
            # 136:776 -- one DMA instead of four on the startup-critical path
            cp_sb = const.tile([128, 776], BF16)
            w1_sb = const.tile([128, 3, 2, 128], BF16)
            w2_sb = const.tile([128, 2, 256], BF16)
            nc.sync.dma_start(out=cp_sb, in_=cpack[:, :])
            embw_sb = cp_sb[:, 0:128]
            sv_sb = cp_sb[:, 128:136].bitcast(F32)

            xyin = [None] * (NT // G)
            tin = [None] * (NT // G)
            # group-0 inputs issue before the (later-needed) w1/w2 weights
            xyin[0] = io.tile([35, G, 512], BF16, tag="xyin", name="xyin0")
            tin[0] = io.tile([128, G, 512], BF16, tag="tin", name="tin0")
            for cc in range(2):
                nc.sync.dma_start(
                    out=xyin[0][32 * cc:32 * cc + 3, :, :],
                    in_=coords[3 * cc:3 * cc + 3, 0:G * 512].rearrange(
                        "p (g n) -> p g n", n=512),
                )
            nc.sync.dma_start(
                out=tin[0],
                in_=tb[:, 0:G * 512].rearrange("p (g n) -> p g n", n=512),
            )
            nc.sync.dma_start(out=w1_sb, in_=w1p[:, :, :, :])
            nc.sync.dma_start(out=w2_sb, in_=w2p[:, :, :])
            hxy = [None] * NT
            ht_ = [None] * NT
            h1T = [None] * NT
            l1ps = [None] * NT
            l2ps = [None] * NT

            for k in range(NT + 2):
                a = k          # stage A: embeds
                b = k - 1      # stage B: layer 1
                c = k - 2      # stage C: layer 2 + store

                if a < NT:
                    ga, ja = divmod(a, G)
                    if ja == 0 and ga > 0:
                        lo, hi = ga * G * 512, (ga + 1) * G * 512
                        xyin[ga] = io.tile([35, G, 512], BF16, tag="xyin", name="xyin")
                        tin[ga] = io.tile([128, G, 512], BF16, tag="tin", name="tin")
                        for cc in range(2):
                            nc.sync.dma_start(
                                out=xyin[ga][32 * cc:32 * cc + 3, :, :],
                                in_=coords[3 * cc:3 * cc + 3, lo:hi].rearrange(
                                    "p (g n) -> p g n", n=512),
                            )
                        nc.sync.dma_start(
                            out=tin[ga],
                            in_=tb[:, lo:hi].rearrange("p (g n) -> p g n", n=512),
                        )

                    # strip matmuls: x-emb(0), y-emb(32), b2 bias(64, 96)
                    if c >= 0:
                        l2ps[c] = ps_l2.tile([128, 4, 256], F32, tag="l2", name="l2ps")
                        nc.tensor.matmul(
                            l2ps[c][:, 0:2, :],
                            cp_sb[96:98, 136:264], cp_sb[96:98, 264:776],
                            start=True, stop=False,
                            skip_group_check=True, tile_position=(96, 0),
                        )
                        nc.tensor.matmul(
                            l2ps[c][:, 2:4, :],
                            cp_sb[64:66, 136:264], cp_sb[64:66, 264:776],
                            start=True, stop=False,
                            skip_group_check=True, tile_position=(64, 0),
                        )
                    emb_ps = ps_emb.tile([128, 2, 512], F32)
                    for cc in range(2):
                        nc.tensor.matmul(
                            emb_ps[:, cc, :],
                            embw_sb[32 * cc:32 * cc + 3, :],
                            xyin[ga][32 * cc:32 * cc + 3, ja, :],
                            start=True, stop=True,
                        )
                    hxy[a] = acts.tile([128, 2, 512], BF16, tag="hxy", name="hxy")
                    nc.scalar.activation(out=hxy[a], in_=emb_ps, func=AF.Sin)
                elif c >= 0:
                    l2ps[c] = ps_l2.tile([128, 4, 256], F32, tag="l2", name="l2ps")
                    for h in range(2):
                        nc.tensor.matmul(
                            l2ps[c][:, 2 * h:2 * h + 2, :],
                            cp_sb[96:98, 136:264], cp_sb[96:98, 264:776],
                            start=True, stop=False,
                            skip_group_check=True, tile_position=(96, 0),
                        )

                # -- stage B: layer 1 (feature-major) -----------------------
                if 0 <= b < NT:
                    l1a = ps_l1a.tile([128, 512], F32, name="l1a")
                    l1b = ps_l1b.tile([128, 512], F32, name="l1b")
                    l1ps[b] = (l1a, l1b)
                    for mc, lp in ((0, l1a), (1, l1b)):
                        for kc in range(2):
                            nc.tensor.matmul(
                                lp,
                                w1_sb[:, kc, mc, :],
                                hxy[b][:, kc, :],
                                start=(kc == 0), stop=False,
                            )
                        nc.tensor.matmul(
                            lp,
                            w1_sb[:, 2, mc, :],
                            ht_[b],
                            start=False, stop=True,
                        )
                    h1T[b] = acts.tile([128, 2, 512], BF16, tag="h1T", name="h1T")
                    # mc1 + mc0-head on DVE, mc0-tail on ACT (engine balance)
                    tmp = acts.tile([128, 512], BF16, tag="tmp1")
                    nc.vector.tensor_scalar_add(
                        out=tmp, in0=l1ps[b][1],
                        scalar1=sv_sb[:, 3:4])
                    nc.vector.scalar_tensor_tensor(
                        out=h1T[b][:, 1, :], in0=tmp, scalar=ALPHA,
                        in1=tmp, op0=ALU.mult, op1=ALU.max)
                    nc.scalar.activation(out=h1T[b][:, 0, :],
                                         in_=l1ps[b][0],
                                         func=AF.Prelu, bias=sv_sb[:, 2:3],
                                         alpha=ALPHA)

                # t-embed on DVE from broadcast tb (issued after mc1 so the
                # l1_ps reader runs at DVE queue head)
                if a < NT:
                    zt = acts.tile([128, 512], BF16, tag="zt")
                    ht_[a] = acts.tile([128, 512], BF16, tag="ht", name="ht")
                    nc.vector.tensor_scalar(
                        out=zt, in0=tin[ga][:, ja, :],
                        scalar1=sv_sb[:, 0:1], scalar2=sv_sb[:, 1:2],
                        op0=ALU.mult, op1=ALU.add)
                    nc.vector.scalar_tensor_tensor(
                        out=ht_[a], in0=zt, scalar=ALPHA, in1=zt,
                        op0=ALU.mult, op1=ALU.max)

                # -- stage C: layer 2 (batch-major) + LeakyReLU + store -----
                if c >= 0:
                    for r in range(4):
                        for kc in range(2):
                            nc.tensor.matmul(
                                l2ps[c][:, r, :],
                                h1T[c][:, kc, r * 128:(r + 1) * 128],
                                w2_sb[:, kc, :],
                                start=False, stop=(kc == 1),
                                skip_group_check=True,
                            )
                    o_sb = outp.tile([128, 4, 256], F32)
                    nc.scalar.activation(out=o_sb, in_=l2ps[c],
                                         func=AF.Prelu, alpha=ALPHA)
                    base = c * TILE
                    nc.sync.dma_start(
                        out=out[base:base + TILE, :].rearrange(
                            "(r p) m -> p r m", p=128),
                        in_=o_sb,
                    )
                    hxy[c] = ht_[c] = h1T[c] = l1ps[c] = l2ps[c] = None
    nc.finalize()
    return nc


def _prep_weights(inputs):
    f = {k: np.asarray(v, dtype=np.float32) for k, v in inputs.items()}
    bf = ml_dtypes.bfloat16

    def hilo(v):
        hi = v.astype(bf).astype(np.float32)
        return hi.astype(bf), (v - hi).astype(bf)

    cpack = np.zeros((128, 776), bf)
    cpack[0, 0:128] = np.concatenate([f["w_sx"].ravel(), f["w_cx"].ravel()])
    bx = np.concatenate([f["b_sx"], f["b_cx"] + math.pi / 2])
    cpack[1, 0:128], cpack[2, 0:128] = hilo(bx)
    cpack[32, 0:128] = np.concatenate([f["w_sy"].ravel(), f["w_cy"].ravel()])
    by = np.concatenate([f["b_sy"], f["b_cy"] + math.pi / 2])
    cpack[33, 0:128], cpack[34, 0:128] = hilo(by)

    svec = np.stack([
        f["w_t"].ravel(),
        f["b_t"],
        f["b1"][0:128],
        f["b1"][128:256],
    ], axis=1)
    svec = np.ascontiguousarray(svec, dtype=np.float32)
    cpack[:, 128:136] = svec.view(np.uint16).view(bf)

    b2hi, b2lo = hilo(f["b2"])
    b2row = np.zeros((2, 640), bf)
    b2row[:, 0:128] = 1.0
    b2row[0, 128:640] = np.concatenate([b2hi, b2hi])
    b2row[1, 128:640] = np.concatenate([b2lo, b2lo])
    cpack[64:66, 136:776] = b2row
    cpack[96:98, 136:776] = b2row

    w1p = f["w1"].reshape(3, 128, 2, 128).transpose(1, 0, 2, 3).astype(bf)
    w2p = f["w2"].reshape(2, 128, 256).transpose(1, 0, 2).astype(bf)

    return {
        "cpack": cpack,
        "w1p": np.ascontiguousarray(w1p),
        "w2p": np.ascontiguousarray(w2p),
    }


def kernel(**inputs):
    if "nc" not in _CACHE:
        _CACHE["nc"] = _build()
    nc = _CACHE["nc"]

    w = _prep_weights(inputs)
    bf = ml_dtypes.bfloat16
    xy = np.asarray(inputs["xy"], dtype=np.float32)
    t = np.asarray(inputs["t"], dtype=np.float32)

    coords = np.empty((6, B), bf)
    coords[0] = xy[:, 0].astype(bf)
    coords[1:3] = 1.0
    coords[3] = xy[:, 1].astype(bf)
    coords[4:6] = 1.0
    t_bf = t[:, 0].astype(bf)

    in_maps = []
    for c in range(NCORES):
        lo, hi = c * R, (c + 1) * R
        in_maps.append({
            "coords": np.ascontiguousarray(coords[:, lo:hi]),
            "tb": np.ascontiguousarray(
                np.broadcast_to(t_bf[lo:hi], (128, R))),
            **w,
        })

    res = run_bass_kernel_spmd(nc, in_maps, core_ids=list(range(NCORES)))
    _CACHE["last_res"] = res
    return np.concatenate([res.results[c]["out"] for c in range(NCORES)], axis=0)
